# revision 34
# baseline (speedup 1.0000x reference)
"""Trainium2 Bass kernel for GPT2-style single attention layer.

Problem: B=4, S=2048, E=1024, H=16 heads, D=64.
  x = hidden @ W_attn + b_attn ; q,k,v = split(x)
  per head: softmax(causal(q k^T / 8) + mask) @ v
  out = merge @ W_proj + b_proj

Sharding over 8 cores: core i -> batch b = i//2, heads h0 = (i%2)*8 .. +8
(data parallel on B, tensor parallel over heads).  Each core's work is fully
local; the host sums the two partial projections per batch.

Dataflow is fully "transposed" so no on-chip transposes are ever needed:
  host feeds xT = hidden[b].T                       [E, S]
  Q^T,K^T = (Wq|Wk block)^T @ xT     -> [d, tok] per head   (W stationary)
  V       = xT_block^T @ Wv          -> [tok, d] natural    (xT stationary)
  S^T     = K^T_blk^T @ Q^T          -> [k, q]   (softmax dim on partitions)
  P^T     = exp(0.125*S^T + mask[k]) * causal01
  sums    = ones^T @ P^T             -> [1, q]  (ones col in V, PSUM-accum)
  attn^T  = V_blk^T @ P^T            -> [d, q]  accumulated over k tiles
  inv     = recip(sums rows 32p, batched) -> gpsimd f32->bf16 cast
  norm    = attn^T * bcast(inv)   (two col-tiled K=1 ones-matmuls per pair)
  out^T   = Wp_blk^T @ attn^T        -> [col, tok]
Host transposes out^T back and sums core pairs + b_proj.

All matmuls run as bf16 (full-rate fp32 path, 1 cycle/row at N>=256).
Startup: ~20 dummy matmuls on a memset tile keep the PE HAM clock-gate warm
while the first DMAs land; the V-critical loads (wv + x quarter 0 in column
chunks) ride the Activation DGE ring, which moves first bytes several us
before the Sync ring spins up.
"""

import os
import ml_dtypes
import numpy as np

B, S, E, H, D = 4, 2048, 1024, 16, 64
NC = 8
HL = H // 2          # local heads per core
EL = HL * D          # local embedding slice = 512
P = 128              # partitions
QT = 512             # q tile width (f32 moving max)
NQT = S // QT        # 4 q tiles
NKT = S // P         # 16 k tiles
NET = E // P         # 8 e (contraction) tiles

_CACHE = {}
LAST_RESULT = None


def _build(has_bv: bool):
    from contextlib import ExitStack

    import concourse.tile as tile
    from concourse import bacc, mybir

    f32 = mybir.dt.float32
    f32r = mybir.dt.bfloat16  # matmul operand dtype (2-byte: full-rate moving operand)
    EXP = mybir.ActivationFunctionType.Exp

    nc = bacc.Bacc(
        "TRN2",
        target_bir_lowering=False,
        debug=False,
        enable_asserts=False,
        num_devices=NC,
    )

    def inp(name, shape, dt=f32):
        return nc.dram_tensor(name, shape, dt, kind="ExternalInput").ap()

    xt_d = inp("xt", [E, S], f32r)
    wq_d = inp("wq", [E, EL], f32r)
    wk_d = inp("wk", [E, EL], f32r)
    wv_d = inp("wv", [E, EL], f32r)
    wp_d = inp("wp", [EL, E], f32r)
    constf_d = inp("constf", [P, 28])          # bq | bk | bv | maskt
    constr_d = inp("constr", [P, 256], f32r)   # causal | ones2
    out_d = nc.dram_tensor("out", [E, S], f32r, kind="ExternalOutput").ap()


    with tile.TileContext(nc) as tc, ExitStack() as ctx:
        const = ctx.enter_context(tc.tile_pool(name="const", bufs=1))
        big = ctx.enter_context(tc.tile_pool(name="big", bufs=1))
        wpool = ctx.enter_context(tc.tile_pool(name="wpool", bufs=1))
        xpool = ctx.enter_context(tc.tile_pool(name="xpool", bufs=1))
        ptpool = ctx.enter_context(tc.tile_pool(name="ptpool", bufs=1))
        aopool = ctx.enter_context(tc.tile_pool(name="aopool", bufs=1))
        ospool = ctx.enter_context(tc.tile_pool(name="ospool", bufs=1))
        rcpool = ctx.enter_context(tc.tile_pool(name="rcpool", bufs=1))
        aospool = ctx.enter_context(tc.tile_pool(name="aospool", bufs=1))
        psum = ctx.enter_context(tc.tile_pool(name="psum", bufs=1, space="PSUM"))


        # ---- PE warm-up: dummy matmuls with no DMA deps keep the HAM
        # clock-gate un-throttled while the first input DMAs are in flight.
        warm = const.tile([P, QT], f32r, name="warm")
        nc.vector.memset(warm[:], 0.5)
        wst = psum.tile([P, 2 * QT], f32, name="wst", tag="st", bufs=2)
        for _ in range(16):
            nc.tensor.matmul(wst[:, 0:QT], warm[:, 0:P], warm[:],
                             start=True, stop=True)

        # ---- persistent big buffers ----
        # Q^T / K^T: per head-pair p a [128, S] tile (partitions = 2 heads x 64 d)
        qt_tiles = [big.tile([P, S], f32r, name=f"qt{p}", tag=f"qt{p}") for p in range(4)]
        kt_tiles = [big.tile([P, S], f32r, name=f"kt{p}", tag=f"kt{p}") for p in range(4)]
        # V natural: 16 tiles [128 tok, 512 vcol]
        v_tiles = [big.tile([P, 8 * 65], f32r, name=f"v{t}", tag=f"v{t}") for t in range(NKT)]

        # consts + the V-critical loads ride the Activation DGE ring: its
        # first bytes land well before the Sync ring's SWDGE spins up.
        constf_t = const.tile([P, 28], f32, name="constf_t")
        nc.scalar.dma_start(constf_t[:], constf_d[:])
        constr_t = const.tile([P, 256], f32r, name="constr_t")
        nc.scalar.dma_start(constr_t[:], constr_d[:])
        bq_t = constf_t[:, 0:4]
        bk_t = constf_t[:, 4:8]
        bv_t = constf_t[:, 8:12]
        maskt_t = constf_t[:, 12:28]
        causal_t = constr_t[:, 0:128]
        ones_t = constr_t[:, 128:256]

        x_tiles = [[None] * NQT for _ in range(NET)]

        def load_x_quarter(tq):
            xb = xpool.tile([P, NET * QT], f32r, name=f"xb{tq}", tag=f"xb{tq}", bufs=1)
            nc.sync.dma_start(
                xb[:].rearrange("p (a c) -> p a c", a=NET, c=QT),
                xt_d.rearrange("(a p) s -> p a s", p=P)[:, :, tq * QT:(tq + 1) * QT],
            )
            for kt in range(NET):
                x_tiles[kt][tq] = xb[:, kt * QT:(kt + 1) * QT]

        def load_w_big(dram, label):
            wb = wpool.tile([P, NET * EL], f32r, name=f"wb_{label}", tag=f"wb_{label}",
                            bufs=1)
            nc.sync.dma_start(
                wb[:].rearrange("p (a c) -> p a c", a=NET, c=EL),
                dram.rearrange("(a p) c -> p a c", p=P),
            )
            return [wb[:, kt * EL:(kt + 1) * EL] for kt in range(NET)]

        # startup (Sync ring — the fast DGE path): wv whole, then x quarter 0
        # in column chunks matched to v_tt's consumption order (tt=0 needs
        # cols 0:128), then the rest ordered by first use.
        wvb = wpool.tile([P, NET * EL], f32r, name="wb_v", tag="wb_v", bufs=1)
        nc.sync.dma_start(
            wvb[:].rearrange("p (a c) -> p a c", a=NET, c=EL),
            wv_d.rearrange("(a p) c -> p a c", p=P),
        )
        wv_t = [wvb[:, kt * EL:(kt + 1) * EL] for kt in range(NET)]
        xb0 = xpool.tile([P, NET * QT], f32r, name="xb0", tag="xb0", bufs=1)
        xv0 = xb0[:].rearrange("p (a c) -> p a c", a=NET, c=QT)
        xs0 = xt_d.rearrange("(a p) s -> p a s", p=P)
        nc.sync.dma_start(xv0[:, :, 0:P], xs0[:, :, 0:P])
        nc.sync.dma_start(xv0[:, :, P:QT], xs0[:, :, P:QT])
        for kt in range(NET):
            x_tiles[kt][0] = xb0[:, kt * QT:(kt + 1) * QT]
        wq_t = load_w_big(wq_d, "q")
        wk_t = load_w_big(wk_d, "k")
        load_x_quarter(1)
        wpb = wpool.tile([P, 4 * E], f32r, name="wpb", tag="wpb", bufs=1)
        nc.sync.dma_start(
            wpb[:].rearrange("p (a c) -> p a c", a=4, c=E),
            wp_d.rearrange("(a p) c -> p a c", p=P),
        )
        wp_tiles = [wpb[:, p * E:(p + 1) * E] for p in range(4)]
        for tq in range(2, NQT):
            load_x_quarter(tq)



        # ---- per-group compute units (run directly or as PE fillers) ----
        done = set()

        def v_tt(tq, tt):
            key = ("v", tq, tt)
            if key in done:
                return
            done.add(key)
            ps = psum.tile([P, EL], f32, name=f"psv{tq}_{tt}", tag="mm", bufs=2)
            for kt in range(NET):
                nc.tensor.matmul(
                    ps[:], x_tiles[kt][tq][:, tt * P:(tt + 1) * P], wv_t[kt][:],
                    start=(kt == 0), stop=(kt == NET - 1))
            vt = v_tiles[tq * 4 + tt]
            v8 = vt[:, 0:520].rearrange("p (a c) -> p a c", a=8, c=65)
            nc.vector.tensor_copy(
                v8[:, :, 0:64], ps[:].rearrange("p (a c) -> p a c", a=8, c=64))
            nc.gpsimd.memset(v8[:, :, 64:65], 1.0)

        def q_ct(tq, ct):
            key = ("q", tq, ct)
            if key in done:
                return
            done.add(key)
            ps = psum.tile([P, QT], f32, name=f"psq{tq}_{ct}", tag="mm", bufs=2)
            for kt in range(NET):
                nc.tensor.matmul(ps[:], wq_t[kt][:, ct * P:(ct + 1) * P],
                                 x_tiles[kt][tq][:],
                                 start=(kt == 0), stop=(kt == NET - 1))
            nc.vector.tensor_scalar_add(
                qt_tiles[ct][:, tq * QT:(tq + 1) * QT], ps[:], bq_t[:, ct:ct + 1])

        def k_ct(tq, ct):
            key = ("k", tq, ct)
            if key in done:
                return
            done.add(key)
            ps = psum.tile([P, QT], f32, name=f"psk{tq}_{ct}", tag="mm", bufs=2)
            for kt in range(NET):
                nc.tensor.matmul(ps[:], wk_t[kt][:, ct * P:(ct + 1) * P],
                                 x_tiles[kt][tq][:],
                                 start=(kt == 0), stop=(kt == NET - 1))
            nc.vector.tensor_scalar_add(
                kt_tiles[ct][:, tq * QT:(tq + 1) * QT], ps[:], bk_t[:, ct:ct + 1])

        def proj_ct(qt, ct, ao_tiles, tail=False):
            key = ("p", qt, ct)
            if key in done:
                return
            done.add(key)
            ps = psum.tile([P, QT], f32, name=f"psp{qt}_{ct}", tag="mm", bufs=2)
            for p in range(4):
                nc.tensor.matmul(ps[:], wp_tiles[p][:, ct * P:(ct + 1) * P],
                                 ao_tiles[p][:], start=(p == 0), stop=(p == 3))
            osb = ospool.tile([P, QT], f32r, name=f"os{qt}_{ct}", tag="os", bufs=2)
            nc.vector.tensor_copy(osb[:], ps[:])
            nc.sync.dma_start(out_d[ct * P:(ct + 1) * P, qt * QT:(qt + 1) * QT],
                              osb[:])

        fillers = []
        late_fillers = []   # reserved for the last quarter's ACT-bound stretch

        def drain_filler(allow_late=False):
            while fillers:
                fn = fillers.pop(0)
                if fn():  # returns True if it actually emitted work
                    return
            if allow_late:
                while late_fillers:
                    fn = late_fillers.pop(0)
                    if fn():
                        return


        def attention(p, qt, sg, allow_late=False):
            """Head pair p (heads 2p, 2p+1), q tile qt.

            Leaves attnout halves in an SBUF tile (f32r) and the packed
            softmax denominators ([1, 2*QT]: head A qs then head B qs) in
            row 32p of sg."""
            kt_max = 4 * (qt + 1)
            # row 64 of each av half accumulates the softmax denominator
            # (ones col); one tile so the recip can read both heads at once
            av2 = psum.tile([65, 2 * QT], f32, name=f"av{p}_{qt}", tag="av", bufs=1)

            def av_sums(kt, pt, off):
                first, last = kt == 0, kt == kt_max - 1
                vva = v_tiles[kt][:, (2 * p) * 65:(2 * p + 1) * 65]
                vvb = v_tiles[kt][:, (2 * p + 1) * 65:(2 * p + 2) * 65]
                nc.tensor.matmul(av2[:, off:QT], vva, pt[:, off:QT],
                                 start=first, stop=last)
                nc.tensor.matmul(av2[:, QT + off:2 * QT], vvb, pt[:, QT + off:2 * QT],
                                 start=first, stop=last)

            pending = None
            for kt in range(kt_max):
                # diagonal tiles: only q columns >= off are unmasked
                diag = kt >= qt * 4
                off = (kt - qt * 4) * P if diag else 0
                kl = slice(kt * P, (kt + 1) * P)
                qv = slice(qt * QT + off, (qt + 1) * QT)
                st = psum.tile([P, 2 * QT], f32, name=f"st{p}_{qt}_{kt}",
                               tag="st", bufs=2)
                nc.tensor.matmul(st[:, off:QT], kt_tiles[p][0:64, kl],
                                 qt_tiles[p][0:64, qv])
                nc.tensor.matmul(st[:, QT + off:2 * QT], kt_tiles[p][64:128, kl],
                                 qt_tiles[p][64:128, qv])
                pt = ptpool.tile([P, 2 * QT], f32r, name=f"pt{p}_{qt}_{kt}",
                                 tag="pt", bufs=5)
                bias = maskt_t[:, kt:kt + 1]
                if not diag or off == 0:
                    nc.scalar.activation(pt[:], st[:], EXP, bias=bias, scale=0.125)
                else:
                    stv = st[:].rearrange("p (h q) -> p h q", h=2, q=QT)[:, :, off:QT]
                    ptv = pt[:].rearrange("p (h q) -> p h q", h=2, q=QT)[:, :, off:QT]
                    nc.scalar.activation(ptv, stv, EXP, bias=bias, scale=0.125)
                if diag:
                    # triangular band at the leading 128 valid columns
                    nc.vector.tensor_mul(pt[:, off:off + P], pt[:, off:off + P],
                                         causal_t[:])
                    nc.vector.tensor_mul(pt[:, QT + off:QT + off + P],
                                         pt[:, QT + off:QT + off + P], causal_t[:])
                if pending is not None:
                    av_sums(*pending)
                    if kt % 2 == 0:
                        drain_filler(allow_late)
                pending = (kt, pt, off)
            av_sums(*pending)

            # sums row first (it gates the recip chain), then the PSUM
            # drain so the next pair's AV can start.  On the last quarter
            # (sg is None) the tail normalize stages the row itself.
            if sg is not None:
                row = 32 * p
                nc.vector.tensor_copy(sg[row:row + 1, :], av2[64:65, :])
            aos = aospool.tile([P, QT], f32r, name=f"aos{p}_{qt}",
                               tag=f"aos{p}", bufs=2)
            nc.vector.tensor_copy(aos[0:64, :], av2[0:64, 0:QT])
            nc.vector.tensor_copy(aos[64:128, :], av2[0:64, QT:2 * QT])
            return aos, av2

        def normalize(qt, rc, aos_tiles):
            """Per-pair softmax normalization: two col-tiled K=1 ones-matmuls
            (concurrent in the PE array) broadcast head A's 1/sums to
            partitions 0:64 and head B's to 64:128 of one PSUM bank, then a
            single full-width multiply."""
            ao_tiles = []
            for p in range(4):
                row = 32 * p
                ao = aopool.tile([P, QT], f32r, name=f"ao{p}_{qt}",
                                 tag=f"ao{p}", bufs=3)
                rb = psum.tile([P, QT], f32, name=f"rb{p}_{qt}", tag="mm",
                               bufs=2)
                nc.tensor.matmul(rb[0:64, :], ones_t[row:row + 1, 0:64],
                                 rc[row:row + 1, 0:QT], tile_position=(row, 0))
                nc.tensor.matmul(rb[64:128, :], ones_t[row:row + 1, 64:128],
                                 rc[row:row + 1, QT:2 * QT],
                                 tile_position=(row, 64))
                nc.vector.tensor_mul(ao[:], rb[:], aos_tiles[p][:])
                if has_bv:
                    nc.vector.tensor_scalar_add(ao[:], ao[:], bv_t[:, p:p + 1])
                ao_tiles.append(ao)
            return ao_tiles

        def normalize_pair_tail(qt, p, aos, av2):
            """Last-quarter per-pair normalize: stage the PSUM ones-row into
            SBUF (the custom-DVE recip misreads PSUM in this kernel context
            and its writes must start at partition 0), recip, gpsimd cast,
            col-tiled broadcast, one multiply.  Keeps the end-of-kernel chain
            short so the PE never idles long enough to re-throttle."""
            sgp = rcpool.tile([1, 2 * QT], f32, name=f"sgp{p}", tag=f"sgp{p}",
                              bufs=1)
            if p == 3:
                # ACT is out of exps by now — staging there runs parallel to
                # the DVE drains, shortening the end-of-kernel chain
                nc.scalar.copy(sgp[0:1, :], av2[64:65, :])
            else:
                nc.vector.tensor_copy(sgp[0:1, :], av2[64:65, :])
            rcp = rcpool.tile([1, 2 * QT], f32, name=f"rcp{p}", tag=f"rcp{p}",
                              bufs=1)
            nc.vector.reciprocal_approx_fast(rcp[0:1, :], sgp[0:1, :])
            rcb = rcpool.tile([1, 2 * QT], f32r, name=f"rcb{p}", tag=f"rcb{p}",
                              bufs=1)
            nc.gpsimd.tensor_copy(rcb[0:1, :], rcp[0:1, :])
            ao = aopool.tile([P, QT], f32r, name=f"aoL{p}", tag=f"ao{p}",
                             bufs=3)
            rb = psum.tile([P, QT], f32, name=f"rbL{p}", tag="mm", bufs=2)
            nc.tensor.matmul(rb[0:64, :], ones_t[0:1, 0:64], rcb[0:1, 0:QT],
                             tile_position=(0, 0))
            nc.tensor.matmul(rb[64:128, :], ones_t[0:1, 64:128],
                             rcb[0:1, QT:2 * QT], tile_position=(0, 64))
            nc.vector.tensor_mul(ao[:], rb[:], aos[:])
            if has_bv:
                nc.vector.tensor_scalar_add(ao[:], ao[:], bv_t[:, p:p + 1])
            return ao


        # ============ filler-queue main schedule ============
        # Attention k-loops are ACT(exp)-paced; PE idle slots are filled with
        # independent matmul groups: next quarter's V/Q/K and deferred proj.
        def mkfiller(fn, *args):
            def run():
                before = len(done)
                fn(*args)
                return len(done) != before
            return run

        pending_np = None
        for tq in range(NQT):
            # mandatory prelude: V + first pair's Q/K; later pairs become
            # fillers drained (or ensured) just in time
            for tt in range(4):
                v_tt(tq, tt)
            q_ct(tq, 0)
            k_ct(tq, 0)
            for ct in range(1, 4):
                fillers.append(mkfiller(q_ct, tq, ct))
                fillers.append(mkfiller(k_ct, tq, ct))
            # queue next quarter's V/Q/K as fillers — except V(0,1)/Q(0)/K(0),
            # which are RESERVED for the next quarter-start prelude: the PE
            # pipeline refills there with no filler coverage (engines execute
            # in order), and an empty prelude idles the PE long enough to
            # trip the HAM re-throttle
            if tq + 1 < NQT:
                for tt in range(2, 4):
                    fillers.append(mkfiller(v_tt, tq + 1, tt))
                for ct in range(1, 4):
                    fillers.append(mkfiller(q_ct, tq + 1, ct))
                    fillers.append(mkfiller(k_ct, tq + 1, ct))
            last = tq == NQT - 1
            if not last:
                sg = rcpool.tile([P, 2 * QT], f32, name=f"sg{tq}", tag="sg",
                                 bufs=2)
                nc.gpsimd.memset(sg[:], 1.0)
                rcf = rcpool.tile([P, 2 * QT], f32, name=f"rcf{tq}",
                                  tag="rcf", bufs=2)
                rc = rcpool.tile([P, 2 * QT], f32r, name=f"rc{tq}", tag="rc",
                                 bufs=2)
            else:
                sg = rc = None
            aos_tiles = []
            ao_last = []
            for p in range(4):
                q_ct(tq, p)
                k_ct(tq, p)
                aos, av2 = attention(p, tq, sg, last)
                if last:
                    # per-pair normalize, pipelined under the next pair's
                    # attention; pair 3's short chain is covered by leftover
                    # late fillers + the proj prefix matmuls
                    ao_last.append(normalize_pair_tail(tq, p, aos, av2))
                else:
                    aos_tiles.append(aos)
                if p == 1 and pending_np is not None:
                    qt_prev = pending_np[0]
                    ao_prev = normalize(*pending_np)
                    # proj(qt0) drains anywhere; proj(qt1)/proj(qt2) are
                    # reserved for the last quarter's ACT-bound attention and
                    # its tail (leftovers run under the final recip chain,
                    # keeping the PE warm so the last proj isn't throttled).
                    dest = fillers if qt_prev == 0 else late_fillers
                    for ct in range(NET):
                        dest.append(mkfiller(proj_ct, qt_prev, ct, ao_prev))
                    pending_np = None
            if not last:
                # one batched reciprocal for all 4 pairs' packed sums rows,
                # then the f32->bf16 cast on the otherwise-idle GPSIMD engine
                nc.vector.reciprocal_approx_fast(rcf[:], sg[:])
                nc.gpsimd.tensor_copy(rc[:], rcf[:])
                pending_np = (tq, rc, aos_tiles)
        # ---- tail: flush fillers (they execute under the last pair's
        # recip/drain chain), then the final proj runs at full clock.
        while fillers:
            fillers.pop(0)()
        while late_fillers:
            late_fillers.pop(0)()
        # always-ready warm matmuls at this FIFO position execute while the
        # last pair's copy/recip/cast/bcast/mul chain runs, so the PE never
        # idles into a re-throttle before the final proj
        junk = psum.tile([P, QT], f32, name="junk", tag="mm", bufs=2)
        for _ in range(18):
            nc.tensor.matmul(junk[:], warm[:, 0:P], warm[:],
                             start=True, stop=True)
        for ct in range(NET):
            proj_ct(NQT - 1, ct, ao_last, tail=True)

    nc.compile()
    return nc


def _causal_tiles():
    """[128, 128] lower-triangular 0/1 band mask (dq >= dk)."""
    dk = np.arange(P)[:, None]
    dq = np.arange(P)[None, :]
    return np.ascontiguousarray((dq >= dk).astype(np.float32))


def kernel(hidden_state, attention_mask, W_attn, b_attn, W_proj, b_proj):
    global LAST_RESULT
    hs = np.asarray(hidden_state, np.float32)
    am = np.asarray(attention_mask, np.float32).reshape(B, S)
    wa = np.asarray(W_attn, np.float32)
    ba = np.asarray(b_attn, np.float32)
    wpr = np.asarray(W_proj, np.float32)
    bp = np.asarray(b_proj, np.float32)

    has_bv = bool(np.any(ba[2 * E:3 * E] != 0.0))
    key = ("k", has_bv)
    if key not in _CACHE:
        _CACHE[key] = _build(has_bv)
    nc = _CACHE[key]

    bf16 = ml_dtypes.bfloat16
    causal = _causal_tiles().astype(bf16)
    constr = np.ascontiguousarray(
        np.concatenate([causal, np.ones((P, P), bf16)], axis=1))
    in_maps = []
    for core in range(NC):
        b = core // 2
        c0 = (core % 2) * EL
        constf = np.ascontiguousarray(np.concatenate(
            [
                ba[c0:c0 + EL].reshape(4, P).T,
                ba[E + c0:E + c0 + EL].reshape(4, P).T,
                ba[2 * E + c0:2 * E + c0 + EL].reshape(4, P).T,
                am[b].reshape(NKT, P).T,
            ],
            axis=1,
        ).astype(np.float32))
        in_maps.append({
            "xt": np.ascontiguousarray(hs[b].T).astype(bf16),
            "wq": np.ascontiguousarray(wa[:, c0:c0 + EL]).astype(bf16),
            "wk": np.ascontiguousarray(wa[:, E + c0:E + c0 + EL]).astype(bf16),
            "wv": np.ascontiguousarray(wa[:, 2 * E + c0:2 * E + c0 + EL]).astype(bf16),
            "wp": np.ascontiguousarray(wpr[c0:c0 + EL, :]).astype(bf16),
            "constf": constf,
            "constr": constr,
        })

    from concourse.bass_utils import run_bass_kernel_spmd

    trace = os.environ.get("KERNEL_TRACE", "") == "1"
    res = run_bass_kernel_spmd(nc, in_maps, core_ids=list(range(NC)), trace=trace)
    LAST_RESULT = res

    full = np.empty((B, S, E), np.float32)
    for b in range(B):
        full[b] = res.results[2 * b]["out"].T.astype(np.float32)
        full[b] += res.results[2 * b + 1]["out"].T.astype(np.float32)
        full[b] += bp
    return full
